# revision 10
# baseline (speedup 1.0000x reference)
"""SeeSaw loss kernel for Trainium2 (8 NeuronCores, batch-parallel).

Math (per batch b, pixel p, with t = target[b,p]):
    M[i,j]     = max(w_i / w_j, 1)
    denom[j,p] = sum_i exp(logit[i,p]) * M[i,j]      (one 128x128 matmul)
    loss_p     = log(denom[t,p]) - logit[t,p]
    out        = mean_p loss_p over all b,h,w

Layout per core (= per batch): classes N=128 on partitions, pixels H*W=16384
along free dim. DVE/ACT/DMA work in 4096-wide groups; matmul+Ln in 2048-wide
PSUM chunks (4 banks, double buffered).

The matmul runs in bf16 (inputs rounded to bf16; fp32 PSUM accumulation).
Per-element bf16 rounding is random across the 128-term contraction and the
131072-pixel mean, so the final loss keeps ~1e-6 relative accuracy.

Selection of the target row uses DVE scalar_tensor_tensor:
    (targ_broadcast == iota_per_partition) * X, accumulated per partition
with X = log(denom) (bf16 from ACT) and X = logit (f32).
Host combines the per-core [128, 8] partial sums (the final mean).
"""

import numpy as np
import ml_dtypes

import concourse.bacc as bacc
import concourse.bass as bass
import concourse.tile as tile
from concourse import mybir

B, N, H, W = 8, 128, 128, 128
HW = H * W
GROUP_WIDTHS = [2048, 4096, 4096, 4096, 2048]  # taper ends for pipeline fill/drain
assert sum(GROUP_WIDTHS) == HW
NGROUP = len(GROUP_WIDTHS)
CW = 2048          # psum chunk width (4 banks)
F32 = mybir.dt.float32
BF16 = mybir.dt.bfloat16

_NC_CACHE = {}


def _patch_act_tables():
    """Make Exp and Ln resolve to the same activation-table set
    (natural_log_exp_and_others) so the table is loaded once instead of
    thrashing between per-function sets on every chunk."""
    import concourse.bacc as _bacc
    from concourse.hw_specs import get_activation_tables as _orig

    def patched(arch):
        # act_func_set_id is the INDEX into this (ordered) dict, so entries
        # must not be removed or reordered -- only membership is edited.
        tabs = dict(_orig(arch))
        E = mybir.ActivationFunctionType.Exp
        L = mybir.ActivationFunctionType.Ln
        for name in ("exp_and_others", "exp_and_friends", "natural_log"):
            if name in tabs:
                tabs[name] = tabs[name] - {E, L}
        return tabs

    _bacc.get_activation_tables = patched


def _build_nc():
    _patch_act_tables()
    nc = bacc.Bacc("TRN2", target_bir_lowering=False)

    logit = nc.dram_tensor("logit", [N, HW], F32, kind="ExternalInput")
    targ = nc.dram_tensor("targ", [1, HW], BF16, kind="ExternalInput")
    # consts packed in one tensor: [:, 0:1]=w col, [:, 1:129]=w row-tiled,
    # [:, 129:130]=iota(bf16 bits in f32 container handled host-side? no --
    # iota kept f32 is fine for the STT scalar operand)
    consts = nc.dram_tensor("consts", [N, 130], F32, kind="ExternalInput")
    acc = nc.dram_tensor("acc", [N, 2 * NGROUP], F32, kind="ExternalOutput")

    with tile.TileContext(nc) as tc:
        with (
            tc.tile_pool(name="singles", bufs=1) as singles,
            tc.tile_pool(name="lg", bufs=3) as lg_pool,
            tc.tile_pool(name="ex", bufs=2) as ex_pool,
            tc.tile_pool(name="ld", bufs=2) as ld_pool,
            tc.tile_pool(name="tb", bufs=2) as tb_pool,
            tc.tile_pool(name="scr", bufs=3) as scr_pool,
            tc.tile_pool(name="psum", bufs=2, space="PSUM") as psum_pool,
        ):
            # first logit chunk DMA issued before anything else
            lg_0 = lg_pool.tile([N, GROUP_WIDTHS[0]], F32, tag="lg0")
            nc.sync.dma_start(out=lg_0[:], in_=logit[:, 0 : GROUP_WIDTHS[0]])

            consts_sb = singles.tile([N, 130], F32)
            nc.sync.dma_start(out=consts_sb[:], in_=consts[:])
            wcol_sb = consts_sb[:, 0:1]
            wrow_sb = consts_sb[:, 1:129]
            iota_sb = consts_sb[:, 129:130]

            # M[i,j] = max(w_i * (1/w_j), 1);  i = partition, j = free.
            recip = singles.tile([N, N], F32)
            m_raw = singles.tile([N, N], F32)
            m_bf = singles.tile([N, N], BF16)
            nc.vector.reciprocal(out=recip[:], in_=wrow_sb)
            nc.vector.tensor_scalar_mul(out=m_raw[:], in0=recip[:], scalar1=wcol_sb)
            nc.vector.tensor_scalar_max(out=m_bf[:], in0=m_raw[:], scalar1=1.0)

            acc_sb = singles.tile([N, 2 * NGROUP], F32)

            goff = 0
            for g, gw in enumerate(GROUP_WIDTHS):
                gsl = slice(goff, goff + gw)
                if g == 0:
                    lg_g = lg_0
                else:
                    lg_g = lg_pool.tile([N, gw], F32, tag=f"lg{min(g,1)}")
                    for c in range(gw // CW):
                        nc.sync.dma_start(
                            out=lg_g[:, c * CW : (c + 1) * CW],
                            in_=logit[:, goff + c * CW : goff + (c + 1) * CW],
                        )

                ex_g = ex_pool.tile([N, gw], BF16, tag="ex")
                ld_g = ld_pool.tile([N, gw], BF16, tag="ld")
                for c in range(gw // CW):
                    csl = slice(c * CW, (c + 1) * CW)
                    nc.scalar.activation(
                        out=ex_g[:, csl],
                        in_=lg_g[:, csl],
                        func=mybir.ActivationFunctionType.Exp,
                    )
                    ps_c = psum_pool.tile([N, CW], F32, tag="ps")
                    for j in range(CW // 512):
                        jsl = slice(c * CW + j * 512, c * CW + (j + 1) * 512)
                        nc.tensor.matmul(
                            ps_c[:, j * 512 : (j + 1) * 512],
                            m_bf[:],
                            ex_g[:, jsl],
                            start=True,
                            stop=True,
                        )
                    nc.scalar.activation(
                        out=ld_g[:, csl],
                        in_=ps_c[:],
                        func=mybir.ActivationFunctionType.Ln,
                    )

                tb_g = tb_pool.tile([N, gw], BF16, tag="tb")
                nc.sync.dma_start(out=tb_g[:], in_=targ[0:1, gsl].to_broadcast([N, gw]))

                s1 = scr_pool.tile([N, gw], BF16, tag="scr")
                nc.vector.scalar_tensor_tensor(
                    out=s1[:],
                    in0=tb_g[:],
                    scalar=iota_sb,
                    in1=ld_g[:],
                    op0=mybir.AluOpType.is_equal,
                    op1=mybir.AluOpType.mult,
                    accum_out=acc_sb[:, 2 * g : 2 * g + 1],
                )
                s2 = scr_pool.tile([N, gw], BF16, tag="scr")
                nc.vector.scalar_tensor_tensor(
                    out=s2[:],
                    in0=tb_g[:],
                    scalar=iota_sb,
                    in1=lg_g[:],
                    op0=mybir.AluOpType.is_equal,
                    op1=mybir.AluOpType.mult,
                    accum_out=acc_sb[:, 2 * g + 1 : 2 * g + 2],
                )
                goff += gw

            nc.sync.dma_start(out=acc[:], in_=acc_sb[:])

    nc.compile()
    return nc


def get_nc():
    if "nc" not in _NC_CACHE:
        _NC_CACHE["nc"] = _build_nc()
    return _NC_CACHE["nc"]


def make_in_maps(logit, target, weight):
    """Shard per batch: core b gets batch b."""
    logit = np.ascontiguousarray(np.asarray(logit, dtype=np.float32))
    target = np.asarray(target)
    weight = np.ascontiguousarray(np.asarray(weight, dtype=np.float32))
    targ_bf = target.astype(np.float32).astype(ml_dtypes.bfloat16)
    iota = np.arange(N, dtype=np.float32)
    in_maps = []
    for b in range(B):
        consts = np.empty((N, 130), dtype=np.float32)
        consts[:, 0] = weight[b]
        consts[:, 1:129] = np.tile(weight[b].reshape(1, N), (N, 1))
        consts[:, 129] = iota
        in_maps.append(
            {
                "logit": logit[b].reshape(N, HW),
                "targ": targ_bf[b].reshape(1, HW),
                "consts": consts,
            }
        )
    return in_maps


def combine(results):
    tot = np.float64(0.0)
    for r in results:
        a = r["acc"].astype(np.float64)
        tot += a[:, 0::2].sum() - a[:, 1::2].sum()
    return np.float32(tot / (B * HW))


def kernel(logit, target, weight, epoch=None, **_ignored):
    from concourse.bass_utils import run_bass_kernel_spmd

    nc = get_nc()
    in_maps = make_in_maps(logit, target, weight)
    res = run_bass_kernel_spmd(nc, in_maps, core_ids=list(range(B)))
    return combine(res.results)


# revision 11
# speedup vs baseline: 1.0871x; 1.0871x over previous
"""SeeSaw loss kernel for Trainium2 (8 NeuronCores, batch-parallel).

Math (per batch b, pixel p, with t = target[b,p]):
    M[i,j]     = max(w_i / w_j, 1)
    denom[j,p] = sum_i exp(logit[i,p]) * M[i,j]      (one 128x128 matmul)
    loss_p     = log(denom[t,p]) - logit[t,p]
    out        = mean_p loss_p over all b,h,w

Layout per core (= per batch): classes N=128 on partitions, pixels H*W=16384
along free dim. DVE/ACT/DMA work in 4096-wide groups; matmul+Ln in 2048-wide
PSUM chunks (4 banks, double buffered).

The matmul runs in bf16 (inputs rounded to bf16; fp32 PSUM accumulation).
Per-element bf16 rounding is random across the 128-term contraction and the
131072-pixel mean, so the final loss keeps ~1e-6 relative accuracy.

Selection of the target row uses DVE scalar_tensor_tensor:
    (targ_broadcast == iota_per_partition) * X, accumulated per partition
with X = log(denom) (bf16 from ACT) and X = logit (f32).
Host combines the per-core [128, 8] partial sums (the final mean).
"""

import numpy as np
import ml_dtypes

import concourse.bacc as bacc
import concourse.bass as bass
import concourse.tile as tile
from concourse import mybir

B, N, H, W = 8, 128, 128, 128
HW = H * W
GROUP_WIDTHS = [4096, 4096, 4096, 4096]
assert sum(GROUP_WIDTHS) == HW
NGROUP = len(GROUP_WIDTHS)
CW = 2048          # psum chunk width (4 banks)
F32 = mybir.dt.float32
BF16 = mybir.dt.bfloat16

_NC_CACHE = {}


def _patch_act_tables():
    """Make Exp and Ln resolve to the same activation-table set
    (natural_log_exp_and_others) so the table is loaded once instead of
    thrashing between per-function sets on every chunk."""
    import concourse.bacc as _bacc
    from concourse.hw_specs import get_activation_tables as _orig

    def patched(arch):
        # act_func_set_id is the INDEX into this (ordered) dict, so entries
        # must not be removed or reordered -- only membership is edited.
        tabs = dict(_orig(arch))
        E = mybir.ActivationFunctionType.Exp
        L = mybir.ActivationFunctionType.Ln
        for name in ("exp_and_others", "exp_and_friends", "natural_log"):
            if name in tabs:
                tabs[name] = tabs[name] - {E, L}
        return tabs

    _bacc.get_activation_tables = patched


def _build_nc():
    _patch_act_tables()
    nc = bacc.Bacc("TRN2", target_bir_lowering=False)

    logit = nc.dram_tensor("logit", [N, HW], F32, kind="ExternalInput")
    targ = nc.dram_tensor("targ", [1, HW], BF16, kind="ExternalInput")
    # consts packed in one tensor: [:, 0:1]=w col, [:, 1:129]=w row-tiled,
    # [:, 129:130]=iota(bf16 bits in f32 container handled host-side? no --
    # iota kept f32 is fine for the STT scalar operand)
    consts = nc.dram_tensor("consts", [N, 130], F32, kind="ExternalInput")
    acc = nc.dram_tensor("acc", [N, 2 * NGROUP], F32, kind="ExternalOutput")

    with tile.TileContext(nc) as tc:
        with (
            tc.tile_pool(name="singles", bufs=1) as singles,
            tc.tile_pool(name="lg", bufs=3) as lg_pool,
            tc.tile_pool(name="ex", bufs=2) as ex_pool,
            tc.tile_pool(name="ld", bufs=2) as ld_pool,
            tc.tile_pool(name="tb", bufs=2) as tb_pool,
            tc.tile_pool(name="scr", bufs=3) as scr_pool,
            tc.tile_pool(name="psum", bufs=2, space="PSUM") as psum_pool,
        ):
            # first logit chunk DMAs issued before anything else
            lg_0 = lg_pool.tile([N, GROUP_WIDTHS[0]], F32, tag="lg0")
            for c in range(GROUP_WIDTHS[0] // CW):
                nc.sync.dma_start(
                    out=lg_0[:, c * CW : (c + 1) * CW],
                    in_=logit[:, c * CW : (c + 1) * CW],
                )

            consts_sb = singles.tile([N, 130], F32)
            nc.sync.dma_start(out=consts_sb[:], in_=consts[:])
            wcol_sb = consts_sb[:, 0:1]
            wrow_sb = consts_sb[:, 1:129]
            iota_sb = consts_sb[:, 129:130]

            # M[i,j] = max(w_i * (1/w_j), 1);  i = partition, j = free.
            recip = singles.tile([N, N], F32)
            m_raw = singles.tile([N, N], F32)
            m_bf = singles.tile([N, N], BF16)
            nc.vector.reciprocal(out=recip[:], in_=wrow_sb)
            nc.vector.tensor_scalar_mul(out=m_raw[:], in0=recip[:], scalar1=wcol_sb)
            nc.vector.tensor_scalar_max(out=m_bf[:], in0=m_raw[:], scalar1=1.0)

            acc_sb = singles.tile([N, 2 * NGROUP], F32)

            goff = 0
            for g, gw in enumerate(GROUP_WIDTHS):
                gsl = slice(goff, goff + gw)
                if g == 0:
                    lg_g = lg_0
                else:
                    lg_g = lg_pool.tile([N, gw], F32, tag=f"lg{min(g,1)}")
                    for c in range(gw // CW):
                        nc.sync.dma_start(
                            out=lg_g[:, c * CW : (c + 1) * CW],
                            in_=logit[:, goff + c * CW : goff + (c + 1) * CW],
                        )

                ex_g = ex_pool.tile([N, gw], BF16, tag="ex")
                ld_g = ld_pool.tile([N, gw], BF16, tag="ld")
                for c in range(gw // CW):
                    csl = slice(c * CW, (c + 1) * CW)
                    nc.scalar.activation(
                        out=ex_g[:, csl],
                        in_=lg_g[:, csl],
                        func=mybir.ActivationFunctionType.Exp,
                    )
                    ps_c = psum_pool.tile([N, CW], F32, tag="ps")
                    for j in range(CW // 512):
                        jsl = slice(c * CW + j * 512, c * CW + (j + 1) * 512)
                        nc.tensor.matmul(
                            ps_c[:, j * 512 : (j + 1) * 512],
                            m_bf[:],
                            ex_g[:, jsl],
                            start=True,
                            stop=True,
                        )
                    nc.scalar.activation(
                        out=ld_g[:, csl],
                        in_=ps_c[:],
                        func=mybir.ActivationFunctionType.Ln,
                    )

                tb_g = tb_pool.tile([N, gw], BF16, tag="tb")
                nc.sync.dma_start(out=tb_g[:], in_=targ[0:1, gsl].to_broadcast([N, gw]))

                s1 = scr_pool.tile([N, gw], BF16, tag="scr")
                nc.vector.scalar_tensor_tensor(
                    out=s1[:],
                    in0=tb_g[:],
                    scalar=iota_sb,
                    in1=ld_g[:],
                    op0=mybir.AluOpType.is_equal,
                    op1=mybir.AluOpType.mult,
                    accum_out=acc_sb[:, 2 * g : 2 * g + 1],
                )
                s2 = scr_pool.tile([N, gw], BF16, tag="scr")
                nc.vector.scalar_tensor_tensor(
                    out=s2[:],
                    in0=tb_g[:],
                    scalar=iota_sb,
                    in1=lg_g[:],
                    op0=mybir.AluOpType.is_equal,
                    op1=mybir.AluOpType.mult,
                    accum_out=acc_sb[:, 2 * g + 1 : 2 * g + 2],
                )
                goff += gw

            nc.sync.dma_start(out=acc[:], in_=acc_sb[:])

    nc.compile()
    return nc


def get_nc():
    if "nc" not in _NC_CACHE:
        _NC_CACHE["nc"] = _build_nc()
    return _NC_CACHE["nc"]


def make_in_maps(logit, target, weight):
    """Shard per batch: core b gets batch b."""
    logit = np.ascontiguousarray(np.asarray(logit, dtype=np.float32))
    target = np.asarray(target)
    weight = np.ascontiguousarray(np.asarray(weight, dtype=np.float32))
    targ_bf = target.astype(np.float32).astype(ml_dtypes.bfloat16)
    iota = np.arange(N, dtype=np.float32)
    in_maps = []
    for b in range(B):
        consts = np.empty((N, 130), dtype=np.float32)
        consts[:, 0] = weight[b]
        consts[:, 1:129] = np.tile(weight[b].reshape(1, N), (N, 1))
        consts[:, 129] = iota
        in_maps.append(
            {
                "logit": logit[b].reshape(N, HW),
                "targ": targ_bf[b].reshape(1, HW),
                "consts": consts,
            }
        )
    return in_maps


def combine(results):
    tot = np.float64(0.0)
    for r in results:
        a = r["acc"].astype(np.float64)
        tot += a[:, 0::2].sum() - a[:, 1::2].sum()
    return np.float32(tot / (B * HW))


def kernel(logit, target, weight, epoch=None, **_ignored):
    from concourse.bass_utils import run_bass_kernel_spmd

    nc = get_nc()
    in_maps = make_in_maps(logit, target, weight)
    res = run_bass_kernel_spmd(nc, in_maps, core_ids=list(range(B)))
    return combine(res.results)


# revision 12
# speedup vs baseline: 1.0888x; 1.0016x over previous
"""SeeSaw loss kernel for Trainium2 (8 NeuronCores, batch-parallel).

Math (per batch b, pixel p, with t = target[b,p]):
    M[i,j]     = max(w_i / w_j, 1)
    denom[j,p] = sum_i exp(logit[i,p]) * M[i,j]      (one 128x128 matmul)
    loss_p     = log(denom[t,p]) - logit[t,p]
    out        = mean_p loss_p over all b,h,w

Layout per core (= per batch): classes N=128 on partitions, pixels H*W=16384
along free dim. DVE/ACT/DMA work in 4096-wide groups; matmul+Ln in 2048-wide
PSUM chunks (4 banks, double buffered).

The matmul runs in bf16 (inputs rounded to bf16; fp32 PSUM accumulation).
Per-element bf16 rounding is random across the 128-term contraction and the
131072-pixel mean, so the final loss keeps ~1e-6 relative accuracy.

Selection of the target row uses DVE scalar_tensor_tensor:
    (targ_broadcast == iota_per_partition) * X, accumulated per partition
with X = log(denom) (bf16 from ACT) and X = logit (f32).
Host combines the per-core [128, 8] partial sums (the final mean).
"""

import numpy as np
import ml_dtypes

import concourse.bacc as bacc
import concourse.bass as bass
import concourse.tile as tile
from concourse import mybir

B, N, H, W = 8, 128, 128, 128
HW = H * W
GROUP_WIDTHS = [4096, 4096, 4096, 4096]
assert sum(GROUP_WIDTHS) == HW
NGROUP = len(GROUP_WIDTHS)
CW = 2048          # psum chunk width (4 banks)
F32 = mybir.dt.float32
BF16 = mybir.dt.bfloat16

_NC_CACHE = {}


def _patch_act_tables():
    """Make Exp and Ln resolve to the same activation-table set
    (natural_log_exp_and_others) so the table is loaded once instead of
    thrashing between per-function sets on every chunk."""
    import concourse.bacc as _bacc
    from concourse.hw_specs import get_activation_tables as _orig

    def patched(arch):
        # act_func_set_id is the INDEX into this (ordered) dict, so entries
        # must not be removed or reordered -- only membership is edited.
        tabs = dict(_orig(arch))
        E = mybir.ActivationFunctionType.Exp
        L = mybir.ActivationFunctionType.Ln
        for name in ("exp_and_others", "exp_and_friends", "natural_log"):
            if name in tabs:
                tabs[name] = tabs[name] - {E, L}
        return tabs

    _bacc.get_activation_tables = patched


def _build_nc():
    _patch_act_tables()
    nc = bacc.Bacc("TRN2", target_bir_lowering=False)

    # chunk-major layout: [chunk, partition, 2048] so each [128, 2048] DMA
    # reads one fully contiguous 1 MiB block from HBM
    logit = nc.dram_tensor("logit", [HW // CW, N, CW], F32, kind="ExternalInput")
    targ = nc.dram_tensor("targ", [1, HW], BF16, kind="ExternalInput")
    # consts packed in one tensor: [:, 0:1]=w col, [:, 1:129]=w row-tiled,
    # [:, 129:130]=iota(bf16 bits in f32 container handled host-side? no --
    # iota kept f32 is fine for the STT scalar operand)
    consts = nc.dram_tensor("consts", [N, 130], F32, kind="ExternalInput")
    acc = nc.dram_tensor("acc", [N, 2 * NGROUP], F32, kind="ExternalOutput")

    with tile.TileContext(nc) as tc:
        with (
            tc.tile_pool(name="singles", bufs=1) as singles,
            tc.tile_pool(name="lg", bufs=3) as lg_pool,
            tc.tile_pool(name="ex", bufs=2) as ex_pool,
            tc.tile_pool(name="ld", bufs=2) as ld_pool,
            tc.tile_pool(name="tb", bufs=2) as tb_pool,
            tc.tile_pool(name="scr", bufs=3) as scr_pool,
            tc.tile_pool(name="psum", bufs=2, space="PSUM") as psum_pool,
        ):
            # first logit chunk DMAs issued before anything else
            lg_0 = lg_pool.tile([N, GROUP_WIDTHS[0]], F32, tag="lg0")
            for c in range(GROUP_WIDTHS[0] // CW):
                nc.sync.dma_start(
                    out=lg_0[:, c * CW : (c + 1) * CW], in_=logit[c]
                )

            consts_sb = singles.tile([N, 130], F32)
            nc.sync.dma_start(out=consts_sb[:], in_=consts[:])
            wcol_sb = consts_sb[:, 0:1]
            wrow_sb = consts_sb[:, 1:129]
            iota_sb = consts_sb[:, 129:130]

            # M[i,j] = max(w_i * (1/w_j), 1);  i = partition, j = free.
            recip = singles.tile([N, N], F32)
            m_raw = singles.tile([N, N], F32)
            m_bf = singles.tile([N, N], BF16)
            nc.vector.reciprocal(out=recip[:], in_=wrow_sb)
            nc.vector.tensor_scalar_mul(out=m_raw[:], in0=recip[:], scalar1=wcol_sb)
            nc.vector.tensor_scalar_max(out=m_bf[:], in0=m_raw[:], scalar1=1.0)

            acc_sb = singles.tile([N, 2 * NGROUP], F32)

            goff = 0
            for g, gw in enumerate(GROUP_WIDTHS):
                gsl = slice(goff, goff + gw)
                if g == 0:
                    lg_g = lg_0
                else:
                    lg_g = lg_pool.tile([N, gw], F32, tag=f"lg{min(g,1)}")
                    for c in range(gw // CW):
                        nc.sync.dma_start(
                            out=lg_g[:, c * CW : (c + 1) * CW],
                            in_=logit[goff // CW + c],
                        )

                ex_g = ex_pool.tile([N, gw], BF16, tag="ex")
                ld_g = ld_pool.tile([N, gw], BF16, tag="ld")
                for c in range(gw // CW):
                    csl = slice(c * CW, (c + 1) * CW)
                    nc.scalar.activation(
                        out=ex_g[:, csl],
                        in_=lg_g[:, csl],
                        func=mybir.ActivationFunctionType.Exp,
                    )
                    ps_c = psum_pool.tile([N, CW], F32, tag="ps")
                    for j in range(CW // 512):
                        jsl = slice(c * CW + j * 512, c * CW + (j + 1) * 512)
                        nc.tensor.matmul(
                            ps_c[:, j * 512 : (j + 1) * 512],
                            m_bf[:],
                            ex_g[:, jsl],
                            start=True,
                            stop=True,
                        )
                    nc.scalar.activation(
                        out=ld_g[:, csl],
                        in_=ps_c[:],
                        func=mybir.ActivationFunctionType.Ln,
                    )

                tb_g = tb_pool.tile([N, gw], BF16, tag="tb")
                nc.sync.dma_start(out=tb_g[:], in_=targ[0:1, gsl].to_broadcast([N, gw]))

                s1 = scr_pool.tile([N, gw], BF16, tag="scr")
                nc.vector.scalar_tensor_tensor(
                    out=s1[:],
                    in0=tb_g[:],
                    scalar=iota_sb,
                    in1=ld_g[:],
                    op0=mybir.AluOpType.is_equal,
                    op1=mybir.AluOpType.mult,
                    accum_out=acc_sb[:, 2 * g : 2 * g + 1],
                )
                s2 = scr_pool.tile([N, gw], BF16, tag="scr")
                nc.vector.scalar_tensor_tensor(
                    out=s2[:],
                    in0=tb_g[:],
                    scalar=iota_sb,
                    in1=lg_g[:],
                    op0=mybir.AluOpType.is_equal,
                    op1=mybir.AluOpType.mult,
                    accum_out=acc_sb[:, 2 * g + 1 : 2 * g + 2],
                )
                goff += gw

            nc.sync.dma_start(out=acc[:], in_=acc_sb[:])

    nc.compile()
    return nc


def get_nc():
    if "nc" not in _NC_CACHE:
        _NC_CACHE["nc"] = _build_nc()
    return _NC_CACHE["nc"]


def make_in_maps(logit, target, weight):
    """Shard per batch: core b gets batch b."""
    logit = np.ascontiguousarray(np.asarray(logit, dtype=np.float32))
    target = np.asarray(target)
    weight = np.ascontiguousarray(np.asarray(weight, dtype=np.float32))
    targ_bf = target.astype(np.float32).astype(ml_dtypes.bfloat16)
    iota = np.arange(N, dtype=np.float32)
    in_maps = []
    for b in range(B):
        consts = np.empty((N, 130), dtype=np.float32)
        consts[:, 0] = weight[b]
        consts[:, 1:129] = np.tile(weight[b].reshape(1, N), (N, 1))
        consts[:, 129] = iota
        in_maps.append(
            {
                "logit": np.ascontiguousarray(
                    logit[b].reshape(N, HW // CW, CW).transpose(1, 0, 2)
                ),
                "targ": targ_bf[b].reshape(1, HW),
                "consts": consts,
            }
        )
    return in_maps


def combine(results):
    tot = np.float64(0.0)
    for r in results:
        a = r["acc"].astype(np.float64)
        tot += a[:, 0::2].sum() - a[:, 1::2].sum()
    return np.float32(tot / (B * HW))


def kernel(logit, target, weight, epoch=None, **_ignored):
    from concourse.bass_utils import run_bass_kernel_spmd

    nc = get_nc()
    in_maps = make_in_maps(logit, target, weight)
    res = run_bass_kernel_spmd(nc, in_maps, core_ids=list(range(B)))
    return combine(res.results)
